# revision 1
# baseline (speedup 1.0000x reference)
"""Trainium2 Bass kernel for nn_AttentionLayer (sparse_attention).

Math (per batch b, history l):
    info = [q, k, q-k, q*k] @ W1 + b1 ; @ W2 + b2 ; sigmoid ; @ Wf + bf
    score = softmax(where(mask, -inf, logit), axis=l)
    out   = sum_l score * v

Host-side algebra (exact up to fp assoc):
  - No nonlinearity between W1 and W2  =>  fold: h2 = k@P + (q*k)@Q + r(q)
        P  = (W1b - W1c) @ W2, Q = W1d @ W2, r = q @ (W1a+W1c)@W2 + b1@W2 + b2
  - sigmoid(x) = 0.5*tanh(x/2) + 0.5  => logit = tanh(h2*0.5) @ (0.5*Wf) + const
    (const cancels in softmax; tanh+exp live in one ACT table set, sigmoid+exp don't)
  - the per-batch bias r is folded into the shipped k / q*k streams:
        solve [P;Q]^T s_b = r_b  (least-norm), ship k + s_b[:64], q*k + s_b[64:]
  - mask => additive -30.0 pre-exp
Device layout: 2 token streams on E-partitions 0:64 / 64:128, batch-pair
chunks of 400 columns; logits land batch-major via psum partition-offset
matmuls + one ACT evacuation + strided SBUF->SBUF DMAs.
"""

import sys

sys.path.insert(0, "/opt/trn_rl_repo")

import numpy as np
import ml_dtypes

import concourse.bass as bass
import concourse.bacc as bacc
import concourse.tile as tile
import concourse.mybir as mybir
from concourse.bass_utils import run_bass_kernel_spmd

N_CORES = 8
B_FULL = 4096
B = B_FULL // N_CORES  # 512 batches per core
L = 200
E = 64
H = 40

NT = (B // 2) * L      # tokens per stream = 51200
CH = 2 * L             # chunk = 2 batches per stream = 400 columns
NCH = NT // CH         # 128 chunks
SLAB_CH = 16           # chunks per DMA slab
NSLAB = NCH // SLAB_CH # 8 slabs
SLAB = SLAB_CH * CH    # 6400 columns

BF16 = mybir.dt.bfloat16
F32 = mybir.dt.float32
nbf16 = ml_dtypes.bfloat16


def build_nc():
    nc = bacc.Bacc()

    kx_d = nc.declare_dram_parameter("kx", [128, NT], BF16, isOutput=False)
    qkx_d = nc.declare_dram_parameter("qkx", [128, NT], BF16, isOutput=False)
    v2_d = nc.declare_dram_parameter("v2", [B, E * L], BF16, isOutput=False)
    madd_d = nc.declare_dram_parameter("madd", [B, L], BF16, isOutput=False)
    pq_d = nc.declare_dram_parameter("pq", [128, 2 * H], BF16, isOutput=False)
    qq_d = nc.declare_dram_parameter("qq", [128, 2 * H], BF16, isOutput=False)
    wf_d = nc.declare_dram_parameter("wf32", [2 * H, 64], BF16, isOutput=False)
    out_d = nc.declare_dram_parameter("out", [B, E], F32, isOutput=True)

    Tanh = mybir.ActivationFunctionType.Tanh
    Exp = mybir.ActivationFunctionType.Exp
    Copy = mybir.ActivationFunctionType.Copy
    Alu = mybir.AluOpType
    X = mybir.AxisListType.X

    from contextlib import ExitStack

    with tile.TileContext(nc) as tc, ExitStack() as ctx:
        const = ctx.enter_context(tc.tile_pool(name="const", bufs=1))
        kxp = ctx.enter_context(tc.tile_pool(name="kxp", bufs=2))
        qkxp = ctx.enter_context(tc.tile_pool(name="qkxp", bufs=2))
        h2p = ctx.enter_context(tc.tile_pool(name="h2p", bufs=3, space="PSUM"))
        lgp = ctx.enter_context(tc.tile_pool(name="lgp", bufs=2, space="PSUM"))
        tp = ctx.enter_context(tc.tile_pool(name="tp", bufs=4))
        lgsp = ctx.enter_context(tc.tile_pool(name="lgsp", bufs=2))
        logp = ctx.enter_context(tc.tile_pool(name="logp", bufs=1))
        vp = ctx.enter_context(tc.tile_pool(name="vp", bufs=2))
        wp = ctx.enter_context(tc.tile_pool(name="wp", bufs=1))
        bp = ctx.enter_context(tc.tile_pool(name="bp", bufs=2))

        # constants
        pq_t = const.tile([128, 2 * H], BF16, tag="pq")
        nc.sync.dma_start(pq_t[:], pq_d[:])
        qq_t = const.tile([128, 2 * H], BF16, tag="qq")
        nc.sync.dma_start(qq_t[:], qq_d[:])
        wf_t = const.tile([2 * H, 64], BF16, tag="wf")
        nc.sync.dma_start(wf_t[:], wf_d[:])

        # one batch-major logit tile per macro; slab-pair 2m,2m+1 fills macro m
        logit_t = [logp.tile([128, L], F32, tag=f"logit{h}", name=f"logit{h}")
                   for h in range(4)]

        # ---------------- Phase B (emitted per-macro, interleaved) ----------
        b_tiles = {}

        def emit_phase_b_loads(m):
            madd_t = bp.tile([128, L], BF16, tag="madd", name=f"madd{m}")
            nc.gpsimd.dma_start(madd_t[:], madd_d[m * 128:(m + 1) * 128, :])
            v_t = vp.tile([128, E * L], BF16, tag="v", name=f"v{m}")
            nc.gpsimd.dma_start(v_t[:], v2_d[m * 128:(m + 1) * 128, :])
            b_tiles[m] = (madd_t, v_t)

        def emit_phase_b(m):
            madd_t, v_t = b_tiles.pop(m)
            lg_view = logit_t[m][:]
            ladj_t = bp.tile([128, L], F32, tag="ladj", name=f"ladj{m}")
            nc.vector.tensor_tensor(ladj_t[:], lg_view, madd_t[:], Alu.add)

            p_t = bp.tile([128, L], BF16, tag="p", name=f"p{m}")
            z_t = bp.tile([128, 1], F32, tag="z", name=f"z{m}")
            nc.scalar.activation(p_t[:], ladj_t[:], Exp, accum_out=z_t[:])

            w_t = wp.tile([128, E * L], BF16, tag="w", name=f"w{m}")
            p_b = p_t[:].rearrange("p (o l) -> p o l", o=1).broadcast_to([128, E, L])
            nc.vector.tensor_tensor(
                w_t[:].rearrange("p (e l) -> p e l", e=E),
                v_t[:].rearrange("p (e l) -> p e l", e=E),
                p_b, Alu.mult,
            )
            # fold l halves at 2x before the 1x reduce
            w2_t = bp.tile([128, E * (L // 2)], BF16, tag="w2", name=f"w2{m}")
            wv = w_t[:].rearrange("p (e l) -> p e l", e=E)
            nc.vector.tensor_tensor(
                w2_t[:].rearrange("p (e l) -> p e l", e=E),
                wv[:, :, 0:L // 2], wv[:, :, L // 2:L], Alu.add,
            )
            acc_t = bp.tile([128, E], F32, tag="acc", name=f"acc{m}")
            nc.vector.tensor_reduce(
                acc_t[:], w2_t[:].rearrange("p (e l) -> p e l", e=E),
                axis=X, op=Alu.add,
            )
            rz_t = bp.tile([128, 1], F32, tag="rz", name=f"rz{m}")
            nc.vector.reciprocal(rz_t[:], z_t[:])
            o_t = bp.tile([128, E], F32, tag="o", name=f"o{m}")
            nc.vector.tensor_scalar_mul(o_t[:], acc_t[:], rz_t[:])
            nc.gpsimd.dma_start(out_d[m * 128:(m + 1) * 128, :], o_t[:])

        # ---------------- Phase A: MLP + tanh + Wf ----------------
        for s in range(NSLAB):
            kx_t = kxp.tile([128, SLAB], BF16, tag="kx", name=f"kx{s}")
            nc.sync.dma_start(kx_t[:], kx_d[:, s * SLAB:(s + 1) * SLAB])
            qkx_t = qkxp.tile([128, SLAB], BF16, tag="qkx", name=f"qkx{s}")
            nc.sync.dma_start(qkx_t[:], qkx_d[:, s * SLAB:(s + 1) * SLAB])

            lgs_t = lgsp.tile([66, SLAB // 2], F32, tag="lgs", name=f"lgs{s}")
            for cc in range(SLAB_CH // 2):  # pair chunks (cc, cc+8)
                h2_t = h2p.tile([80, 1024], F32, tag="h2", name=f"h2_{s}_{cc}")
                lg_t = lgp.tile([128, 512], F32, tag="lg", name=f"lg_{s}_{cc}")
                for j in range(2):
                    col = (cc + 8 * j) * CH
                    rk = kx_t[:, col:col + CH]
                    rq = qkx_t[:, col:col + CH]
                    o = h2_t[0:80, j * 512:j * 512 + CH]
                    nc.tensor.matmul(o, pq_t[:], rk, start=True, stop=False)
                    nc.tensor.matmul(o, qq_t[:], rq, start=False, stop=True)
                t_t = tp.tile([80, 2 * CH], BF16, tag="t", name=f"t_{s}_{cc}")
                nc.scalar.activation(
                    t_t[:].rearrange("p (j c) -> p j c", j=2),
                    h2_t[0:80].rearrange("p (j c) -> p j c", j=2)[:, :, 0:CH],
                    Tanh, scale=0.5,
                )
                for j in range(2):
                    # chunk (cc + 8j) logits -> psum partitions {64j, 64j+1}
                    nc.tensor.matmul(
                        lg_t[64 * j:64 * j + 64, 0:CH],
                        wf_t[:], t_t[:, j * CH:(j + 1) * CH],
                        start=True, stop=True,
                    )
                # evacuate both chunks' logits into the slab staging tile
                nc.scalar.activation(
                    lgs_t[:, cc * CH:(cc + 1) * CH], lg_t[0:66, 0:CH], Copy)
            # 4 DMAs/slab into macro tile (s//2), half (s%2):
            # lgs row 0  = A-batches of this slab      -> macro rows +0:16
            # lgs row 64 = A-batches +16               -> macro rows +16:32
            # lgs row 1  = B-batches                   -> macro rows +32:48
            # lgs row 65 = B-batches +16               -> macro rows +48:64
            mt = logit_t[s // 2]
            pb = 64 * (s % 2)
            for j in range(2):
                nc.sync.dma_start(mt[pb + 16 * j:pb + 16 * j + 16, :],
                                  lgs_t[64 * j:64 * j + 1, :])
                nc.sync.dma_start(mt[pb + 32 + 16 * j:pb + 32 + 16 * j + 16, :],
                                  lgs_t[64 * j + 1:64 * j + 2, :])

            if s % 2 == 0:   # prefetch next macro's v/mask during odd slab
                emit_phase_b_loads(s // 2)
            else:            # macro s//2 logits complete
                emit_phase_b(s // 2)

    if not nc.is_finalized():
        nc.finalize()
    return nc


def host_prep(q, k, v, mask, W1, b1, W2, b2, Wf, bf):
    """Fold weights, build per-core device input maps."""
    q2 = q[:, 0, :].astype(np.float32)                      # [B,64]
    W1 = W1.astype(np.float32); W2 = W2.astype(np.float32)
    P = (W1[64:128] - W1[128:192]) @ W2                     # [64,40]
    Q = W1[192:256] @ W2                                    # [64,40]
    A2 = (W1[0:64] + W1[128:192]) @ W2                      # [64,40]
    c0 = b1.astype(np.float32) @ W2 + b2.astype(np.float32) # [40]
    r = q2 @ A2 + c0                                        # [B,40]
    M = np.concatenate([P, Q], axis=0)                      # [128,40]
    # least-norm s with M^T s = r  ->  s = M (M^T M)^-1 r
    G = M.T @ M
    S = r @ np.linalg.solve(G, M.T).astype(np.float32)      # [B,128]

    kq = q[:, :, :] * k                                     # [B,L,64]
    kb = k + S[:, None, 0:64]
    qkb = kq + S[:, None, 64:128]

    pq = np.zeros((128, 2 * H), np.float32)
    pq[0:64, 0:H] = P; pq[64:128, H:2 * H] = P
    qq = np.zeros((128, 2 * H), np.float32)
    qq[0:64, 0:H] = Q; qq[64:128, H:2 * H] = Q
    wf32 = np.zeros((2 * H, 64), np.float32)
    wf32[0:H, 0] = 0.5 * Wf[:, 0]; wf32[H:2 * H, 1] = 0.5 * Wf[:, 0]

    pq = pq.astype(nbf16); qq = qq.astype(nbf16); wf32 = wf32.astype(nbf16)
    maddf = np.where(mask[:, :, 0], np.float32(-30.0), np.float32(0.0)).astype(nbf16)

    # stream-position -> global-batch maps: slab-pair 2m,2m+1 carries macro m
    gA = np.empty(B // 2, np.int64)
    gB = np.empty(B // 2, np.int64)
    for s in range(8):
        g0 = 128 * (s // 2) + 64 * (s % 2)
        gA[32 * s:32 * s + 32] = g0 + np.arange(32)
        gB[32 * s:32 * s + 32] = g0 + 32 + np.arange(32)

    in_maps = []
    for c in range(N_CORES):
        sl = slice(c * B, (c + 1) * B)
        kbl, qkbl = kb[sl], qkb[sl]
        kx = np.concatenate([kbl[gA].reshape(NT, E).T,
                             kbl[gB].reshape(NT, E).T], axis=0)
        qkx = np.concatenate([qkbl[gA].reshape(NT, E).T,
                              qkbl[gB].reshape(NT, E).T], axis=0)
        v2 = np.ascontiguousarray(v[sl].transpose(0, 2, 1)).reshape(B, E * L)
        in_maps.append({
            "kx": np.ascontiguousarray(kx).astype(nbf16),
            "qkx": np.ascontiguousarray(qkx).astype(nbf16),
            "v2": v2.astype(nbf16),
            "madd": np.ascontiguousarray(maddf[sl]),
            "pq": pq, "qq": qq, "wf32": wf32,
        })
    return in_maps


_CACHE = {}


def run_on_device(in_maps, trace=False):
    if "nc" not in _CACHE:
        _CACHE["nc"] = build_nc()
    nc = _CACHE["nc"]
    res = run_bass_kernel_spmd(nc, in_maps, core_ids=list(range(N_CORES)),
                               trace=trace)
    return res


def kernel(q, k, v, mask, W1, b1, W2, b2, Wf, bf):
    in_maps = host_prep(q, k, v, mask, W1, b1, W2, b2, Wf, bf)
    res = run_on_device(in_maps)
    out = np.concatenate([res.results[c]["out"] for c in range(N_CORES)], axis=0)
    return out.astype(np.float32)



# revision 4
# speedup vs baseline: 1.3408x; 1.3408x over previous
"""Trainium2 Bass kernel for nn_AttentionLayer (sparse_attention).

Math (per batch b, history l):
    info = [q, k, q-k, q*k] @ W1 + b1 ; @ W2 + b2 ; sigmoid ; @ Wf + bf
    score = softmax(where(mask, -inf, logit), axis=l)
    out   = sum_l score * v

Host-side algebra (exact up to fp assoc):
  - No nonlinearity between W1/W2  =>  h2 = k@P + (q*k)@Q + r_b
        P = (W1b-W1c)@W2, Q = W1d@W2, r_b = q_b@(W1a+W1c)@W2 + b1@W2 + b2
  - Fold q into per-batch weights: h2 = k @ V_b + r_b,  V_b = P + diag(q_b) Q
  - Fold r_b into k: solve s_b @ V_b = r_b (least-norm), ship k + s_b
  - sigmoid(x)@Wf = tanh(x/2)@(Wf/2) + const; const cancels in softmax
  - MASK COMPACTION: masked tokens (exp(-inf)=0) are dropped on host; each
    batch's <=126 unmasked tokens are packed into 128 slots (pads: k=0 ->
    logit 0, madd=-30, v=0). Halves k/v traffic and all device compute.
Device layout: token-major 2-stream columns (batch-pair r -> 128 cols,
partitions 0:64 = stream-A E-dims, 64:128 = stream-B). One block-diagonal
[128,80] matmul per pair -> h2 [80,128] (A h2 parts 0:40, B 40:80); tanh
(scale .5) -> t bf16; wf matmuls [80,2] write logits into 4 PSUM partition
strips (32s, 32s+1) at N=512; ACT-copy evacuates [98,512] to bf16 staging;
8 strided DMAs per quarter land logits batch-major [128,128]; softmax + p@v
on DVE (exp w/ accum z on ACT; mult + 2 folds + reduce + scale).
"""

import sys

sys.path.insert(0, "/opt/trn_rl_repo")

import numpy as np
import ml_dtypes

import concourse.bass as bass
import concourse.bacc as bacc
import concourse.tile as tile
import concourse.mybir as mybir
from concourse.bass_utils import run_bass_kernel_spmd

N_CORES = 8
B_FULL = 4096
B = B_FULL // N_CORES  # 512 batches per core
E = 64
H = 40
LP = 128               # compacted history slots per batch
NPAIR = B // 2         # 256 batch pairs per core
NGRP = 16              # pairs per group (one h2 psum tile)
NSLAB = 8              # kx/vw DMA slabs (32 pairs each)

BF16 = mybir.dt.bfloat16
F32 = mybir.dt.float32
nbf16 = ml_dtypes.bfloat16


def build_nc():
    nc = bacc.Bacc()

    kx_d = nc.declare_dram_parameter("kx", [128, NPAIR * LP], BF16, isOutput=False)
    vw_d = nc.declare_dram_parameter("vw", [128, NPAIR * 80], BF16, isOutput=False)
    wf_d = nc.declare_dram_parameter("wf2", [80, 2], BF16, isOutput=False)
    vt_d = nc.declare_dram_parameter("vt", [B, E * LP], BF16, isOutput=False)
    madd_d = nc.declare_dram_parameter("madd", [B, LP], BF16, isOutput=False)
    out_d = nc.declare_dram_parameter("out", [B, E], F32, isOutput=True)

    Tanh = mybir.ActivationFunctionType.Tanh
    Exp = mybir.ActivationFunctionType.Exp
    Copy = mybir.ActivationFunctionType.Copy
    Alu = mybir.AluOpType
    X = mybir.AxisListType.X

    SLABC = 2 * NGRP * LP   # kx cols per slab (4096)
    SLABW = 2 * NGRP * 80   # vw cols per slab (2560)

    from contextlib import ExitStack

    with tile.TileContext(nc) as tc, ExitStack() as ctx:
        const = ctx.enter_context(tc.tile_pool(name="const", bufs=1))
        kxp = ctx.enter_context(tc.tile_pool(name="kxp", bufs=2))
        vwp = ctx.enter_context(tc.tile_pool(name="vwp", bufs=2))
        h2p = ctx.enter_context(tc.tile_pool(name="h2p", bufs=2, space="PSUM"))
        lgp = ctx.enter_context(tc.tile_pool(name="lgp", bufs=2, space="PSUM"))
        tp = ctx.enter_context(tc.tile_pool(name="tp", bufs=2))
        stp = ctx.enter_context(tc.tile_pool(name="stp", bufs=2))
        lmp = ctx.enter_context(tc.tile_pool(name="lmp", bufs=2))
        vtp = ctx.enter_context(tc.tile_pool(name="vtp", bufs=2))
        mp = ctx.enter_context(tc.tile_pool(name="mp", bufs=2))
        bp = ctx.enter_context(tc.tile_pool(name="bp", bufs=2))

        wf_t = const.tile([80, 2], BF16, tag="wf")
        nc.sync.dma_start(wf_t[:], wf_d[:])

        kx_t = {}
        vw_t = {}

        def load_slab(s):
            kx_t[s] = kxp.tile([128, SLABC], BF16, tag="kx", name=f"kx{s}")
            nc.sync.dma_start(kx_t[s][:], kx_d[:, s * SLABC:(s + 1) * SLABC])
            vw_t[s] = vwp.tile([128, SLABW], BF16, tag="vw", name=f"vw{s}")
            nc.sync.dma_start(vw_t[s][:], vw_d[:, s * SLABW:(s + 1) * SLABW])

        qdat = {}

        def load_quarter(qq):
            vt_t = vtp.tile([128, E * LP], BF16, tag="vt", name=f"vt{qq}")
            nc.gpsimd.dma_start(vt_t[:], vt_d[qq * 128:(qq + 1) * 128, :])
            md_t = mp.tile([128, LP], BF16, tag="md", name=f"md{qq}")
            nc.gpsimd.dma_start(md_t[:], madd_d[qq * 128:(qq + 1) * 128, :])
            qdat[qq] = (vt_t, md_t)

        def phase_b(qq, lm_t):
            vt_t, md_t = qdat.pop(qq)
            ladj = bp.tile([128, LP], F32, tag="ladj", name=f"ladj{qq}")
            nc.vector.tensor_tensor(ladj[:], lm_t[:], md_t[:], Alu.add)
            p_t = bp.tile([128, LP], BF16, tag="p", name=f"p{qq}")
            z_t = bp.tile([128, 1], F32, tag="z", name=f"z{qq}")
            nc.scalar.activation(p_t[:], ladj[:], Exp, accum_out=z_t[:])

            w1 = bp.tile([128, E * LP], BF16, tag="w1", name=f"w1{qq}")
            p_b = p_t[:].rearrange("p (o l) -> p o l", o=1).broadcast_to([128, E, LP])
            nc.vector.tensor_tensor(
                w1[:].rearrange("p (e l) -> p e l", e=E),
                vt_t[:].rearrange("p (e l) -> p e l", e=E),
                p_b, Alu.mult,
            )
            w2 = bp.tile([128, E * LP // 2], BF16, tag="w2", name=f"w2{qq}")
            w1v = w1[:].rearrange("p (e l) -> p e l", e=E)
            nc.vector.tensor_tensor(
                w2[:].rearrange("p (e l) -> p e l", e=E),
                w1v[:, :, 0:LP // 2], w1v[:, :, LP // 2:LP], Alu.add,
            )
            w3 = bp.tile([128, E * LP // 4], BF16, tag="w3", name=f"w3{qq}")
            w2v = w2[:].rearrange("p (e l) -> p e l", e=E)
            nc.vector.tensor_tensor(
                w3[:].rearrange("p (e l) -> p e l", e=E),
                w2v[:, :, 0:LP // 4], w2v[:, :, LP // 4:LP // 2], Alu.add,
            )
            acc = bp.tile([128, E], F32, tag="acc", name=f"acc{qq}")
            nc.vector.tensor_reduce(
                acc[:], w3[:].rearrange("p (e l) -> p e l", e=E), axis=X, op=Alu.add)
            rz = bp.tile([128, 1], F32, tag="rz", name=f"rz{qq}")
            nc.vector.reciprocal(rz[:], z_t[:])
            o_t = bp.tile([128, E], F32, tag="o", name=f"o{qq}")
            nc.vector.tensor_scalar_mul(o_t[:], acc[:], rz[:])
            nc.gpsimd.dma_start(out_d[qq * 128:(qq + 1) * 128, :], o_t[:])

        load_slab(0)
        st_t = None
        lg_t = None
        GP = 8  # pairs per h2 group ([80, 1024] f32 = 2 psum banks)
        for g in range(32):
            s = g // 4
            if g % 4 == 0 and s + 1 < NSLAB:
                load_slab(s + 1)
            if g == 0:
                load_quarter(0)
            if g % 8 == 4 and g // 8 + 1 < 4:
                load_quarter(g // 8 + 1)

            kxs, vws = kx_t[s], vw_t[s]
            h2_t = h2p.tile([80, GP * LP], F32, tag="h2", name=f"h2_{g}")
            for pp in range(GP):
                rr = (g % 4) * GP + pp  # pair within slab
                nc.tensor.matmul(
                    h2_t[0:80, pp * LP:(pp + 1) * LP],
                    vws[:, rr * 80:rr * 80 + 80],
                    kxs[:, rr * LP:(rr + 1) * LP],
                    start=True, stop=True,
                )
            t_t = tp.tile([80, GP * LP], BF16, tag="t", name=f"t_{g}")
            nc.scalar.activation(t_t[:], h2_t[:], Tanh, scale=0.5)

            if g % 2 == 0:
                lg_t = lgp.tile([98, 512], F32, tag="lg", name=f"lg_{g // 2}")
            for j in range(2):
                ss = 2 * (g % 2) + j
                nc.tensor.matmul(
                    lg_t[32 * ss:32 * ss + 2, 0:512],
                    wf_t[:], t_t[:, 512 * j:512 * (j + 1)],
                    start=True, stop=True, tile_position=(0, 32 * ss),
                )
            if g % 2 == 1:
                qq, gq = g // 8, (g // 2) % 4
                if gq == 0:
                    st_t = stp.tile([98, 4 * 512], BF16, tag="st", name=f"st{qq}")
                nc.scalar.activation(
                    st_t[:, 512 * gq:512 * (gq + 1)], lg_t[:], Copy)

                if gq == 3:
                    lm_t = lmp.tile([128, LP], BF16, tag="lm", name=f"lm{qq}")
                    for ss in range(4):
                        for sig in range(2):
                            row = 32 * ss + sig
                            dr = 16 * (2 * ss + sig)
                            nc.gpsimd.dma_start(
                                lm_t[dr:dr + 16, :], st_t[row:row + 1, :])
                    phase_b(qq, lm_t)

    if not nc.is_finalized():
        nc.finalize()
    return nc


def host_prep(q, k, v, mask, W1, b1, W2, b2, Wf, bf):
    """Fold weights per batch, compact masked tokens, build device layouts."""
    q2 = q[:, 0, :].astype(np.float32)
    W1 = W1.astype(np.float32); W2 = W2.astype(np.float32)
    P = (W1[64:128] - W1[128:192]) @ W2                     # [64,40]
    Q = W1[192:256] @ W2                                    # [64,40]
    A2 = (W1[0:64] + W1[128:192]) @ W2
    c0 = b1.astype(np.float32) @ W2 + b2.astype(np.float32)
    r = q2 @ A2 + c0                                        # [Bf,40]
    V = P[None] + q2[:, :, None] * Q[None]                  # [Bf,64,40]
    G = np.einsum('beh,bei->bhi', V, V)
    y = np.linalg.solve(G, r[:, :, None])
    s = np.einsum('beh,bhx->be', V, y)                      # [Bf,64]

    m = mask[:, :, 0]
    order = np.argsort(m, axis=1, kind='stable')[:, :LP]
    nvalid = (~m).sum(1)
    assert nvalid.max() <= LP, f"batch with {nvalid.max()} unmasked tokens"
    validc = np.arange(LP)[None, :] < nvalid[:, None]       # [Bf,LP]
    kc = np.take_along_axis(k.astype(np.float32), order[:, :, None], 1)
    vc = np.take_along_axis(v.astype(np.float32), order[:, :, None], 1)
    kc = np.where(validc[..., None], kc + s[:, None, :], 0.0)
    vc = np.where(validc[..., None], vc, 0.0)
    maddf = np.where(validc, np.float32(0.0), np.float32(-30.0)).astype(nbf16)

    # core-local batch <-> (pair r, stream sig) map
    b = np.arange(B)
    qq = b // 128; t = b % 128
    s2s = t // 16; s_ = s2s // 2; sig = s2s % 2
    g_ = (t % 16) // 4; cb = t % 4
    r_ = 64 * qq + 16 * g_ + 4 * s_ + cb
    A_idx = np.empty(NPAIR, np.int64); B_idx = np.empty(NPAIR, np.int64)
    A_idx[r_[sig == 0]] = b[sig == 0]
    B_idx[r_[sig == 1]] = b[sig == 1]

    in_maps = []
    for c in range(N_CORES):
        sl = slice(c * B, (c + 1) * B)
        kcc, Vc = kc[sl], V[sl]
        kx = np.empty((128, NPAIR * LP), np.float32)
        kx[0:64] = kcc[A_idx].transpose(2, 0, 1).reshape(64, -1)
        kx[64:128] = kcc[B_idx].transpose(2, 0, 1).reshape(64, -1)
        vw3 = np.zeros((NPAIR, 128, 80), np.float32)
        vw3[:, 0:64, 0:40] = Vc[A_idx]
        vw3[:, 64:128, 40:80] = Vc[B_idx]
        vw = vw3.transpose(1, 0, 2).reshape(128, NPAIR * 80)
        vt = np.ascontiguousarray(vc[sl].transpose(0, 2, 1)).reshape(B, E * LP)
        wf2 = np.zeros((80, 2), np.float32)
        wf2[0:40, 0] = 0.5 * Wf[:, 0]
        wf2[40:80, 1] = 0.5 * Wf[:, 0]
        in_maps.append({
            "kx": np.ascontiguousarray(kx).astype(nbf16),
            "vw": np.ascontiguousarray(vw).astype(nbf16),
            "wf2": wf2.astype(nbf16),
            "vt": vt.astype(nbf16),
            "madd": np.ascontiguousarray(maddf[sl]),
        })
    return in_maps


_CACHE = {}


def run_on_device(in_maps, trace=False):
    if "nc" not in _CACHE:
        _CACHE["nc"] = build_nc()
    nc = _CACHE["nc"]
    res = run_bass_kernel_spmd(nc, in_maps, core_ids=list(range(N_CORES)),
                               trace=trace)
    return res


def kernel(q, k, v, mask, W1, b1, W2, b2, Wf, bf):
    in_maps = host_prep(q, k, v, mask, W1, b1, W2, b2, Wf, bf)
    res = run_on_device(in_maps)
    out = np.concatenate([res.results[c]["out"] for c in range(N_CORES)], axis=0)
    return out.astype(np.float32)


# revision 8
# speedup vs baseline: 1.5228x; 1.1357x over previous
"""Trainium2 Bass kernel for nn_AttentionLayer (sparse_attention).

Math (per batch b, history l):
    info = [q, k, q-k, q*k] @ W1 + b1 ; @ W2 + b2 ; sigmoid ; @ Wf + bf
    score = softmax(where(mask, -inf, logit), axis=l)
    out   = sum_l score * v

Host-side algebra (exact up to fp assoc):
  - No nonlinearity between W1/W2  =>  h2 = k@P + (q*k)@Q + r_b
        P = (W1b-W1c)@W2, Q = W1d@W2, r_b = q_b@(W1a+W1c)@W2 + b1@W2 + b2
  - Fold q into per-batch weights: h2 = k @ V_b + r_b,  V_b = P + diag(q_b) Q
  - Fold r_b into k: solve s_b @ V_b = r_b (least-norm), ship k + s_b
  - sigmoid(x)@Wf = tanh(x/2)@(Wf/2) + const; const cancels in softmax
  - MASK COMPACTION: masked tokens (exp(-inf)=0) are dropped on host; each
    batch's <=126 unmasked tokens are packed into 128 slots (pads: k=0 ->
    logit 0, madd=-30, v=0). Halves k/v traffic and all device compute.
Device layout: token-major 2-stream columns (batch-pair r -> 128 cols,
partitions 0:64 = stream-A E-dims, 64:128 = stream-B). One block-diagonal
[128,80] matmul per pair -> h2 [80,128] (A h2 parts 0:40, B 40:80); tanh
(scale .5) -> t bf16; wf matmuls [80,2] write logits into 4 PSUM partition
strips (32s, 32s+1) at N=512; ACT-copy evacuates [98,512] to bf16 staging;
8 strided DMAs per quarter land logits batch-major [128,128]; softmax + p@v
on DVE (exp w/ accum z on ACT; mult + 2 folds + reduce + scale).
"""

import sys

sys.path.insert(0, "/opt/trn_rl_repo")

import numpy as np
import ml_dtypes

import concourse.bass as bass
import concourse.bacc as bacc
import concourse.tile as tile
import concourse.mybir as mybir
from concourse.bass_utils import run_bass_kernel_spmd

N_CORES = 8
B_FULL = 4096
B = B_FULL // N_CORES  # 512 batches per core
E = 64
H = 40
LP = 128               # compacted history slots per batch
NPAIR = B // 2         # 256 batch pairs per core
NGRP = 16              # pairs per group (one h2 psum tile)
NSLAB = 8              # kx/vw DMA slabs (32 pairs each)

BF16 = mybir.dt.bfloat16
F32 = mybir.dt.float32
nbf16 = ml_dtypes.bfloat16


def build_nc():
    nc = bacc.Bacc()

    kx_d = nc.declare_dram_parameter("kx", [128, NPAIR * LP], BF16, isOutput=False)
    vw_d = nc.declare_dram_parameter("vw", [128, NPAIR * 80], BF16, isOutput=False)
    wf_d = nc.declare_dram_parameter("wf2", [80, 2], BF16, isOutput=False)
    vt_d = nc.declare_dram_parameter("vt", [B, E * LP], BF16, isOutput=False)
    madd_d = nc.declare_dram_parameter("madd", [B, LP], BF16, isOutput=False)
    out_d = nc.declare_dram_parameter("out", [B, E], F32, isOutput=True)

    Tanh = mybir.ActivationFunctionType.Tanh
    Exp = mybir.ActivationFunctionType.Exp
    Copy = mybir.ActivationFunctionType.Copy
    Alu = mybir.AluOpType
    X = mybir.AxisListType.X

    SLABC = 2 * NGRP * LP   # kx cols per slab (4096)
    SLABW = 2 * NGRP * 80   # vw cols per slab (2560)

    from contextlib import ExitStack

    with tile.TileContext(nc) as tc, ExitStack() as ctx:
        const = ctx.enter_context(tc.tile_pool(name="const", bufs=1))
        kxp = ctx.enter_context(tc.tile_pool(name="kxp", bufs=2))
        vwp = ctx.enter_context(tc.tile_pool(name="vwp", bufs=2))
        h2p = ctx.enter_context(tc.tile_pool(name="h2p", bufs=2, space="PSUM"))
        lgp = ctx.enter_context(tc.tile_pool(name="lgp", bufs=2, space="PSUM"))
        tp = ctx.enter_context(tc.tile_pool(name="tp", bufs=2))
        stp = ctx.enter_context(tc.tile_pool(name="stp", bufs=2))
        lmp = ctx.enter_context(tc.tile_pool(name="lmp", bufs=2))
        vtp = ctx.enter_context(tc.tile_pool(name="vtp", bufs=2))
        mp = ctx.enter_context(tc.tile_pool(name="mp", bufs=2))
        bp = ctx.enter_context(tc.tile_pool(name="bp", bufs=2))

        wf_t = const.tile([80, 2], BF16, tag="wf")
        nc.sync.dma_start(wf_t[:], wf_d[:])

        kx_t = {}
        vw_t = {}

        # variable slabs (in groups of 8 pairs): small first slabs so the
        # first matmul starts after ~256KB of DMA instead of 1.6MB
        SLAB_GROUPS = [1, 1, 2, 4, 4, 4, 4, 4, 4, 4]
        SLAB_G0 = np.cumsum([0] + SLAB_GROUPS).tolist()

        def load_slab(s):
            ng = SLAB_GROUPS[s]
            g0 = SLAB_G0[s]
            kt = kxp.tile([128, 4 * 8 * LP], BF16, tag="kx", name=f"kx{s}")
            nc.sync.dma_start(kt[:, 0:ng * 8 * LP],
                              kx_d[:, g0 * 8 * LP:(g0 + ng) * 8 * LP])
            kx_t[s] = kt
            wt = vwp.tile([128, 4 * 8 * 80], BF16, tag="vw", name=f"vw{s}")
            nc.sync.dma_start(wt[:, 0:ng * 8 * 80],
                              vw_d[:, g0 * 8 * 80:(g0 + ng) * 8 * 80])
            vw_t[s] = wt

        qdat = {}

        def load_quarter(qq):
            vt_t = vtp.tile([128, E * LP], BF16, tag="vt", name=f"vt{qq}")
            nc.gpsimd.dma_start(vt_t[:], vt_d[qq * 128:(qq + 1) * 128, :])
            md_t = mp.tile([128, LP], BF16, tag="md", name=f"md{qq}")
            nc.gpsimd.dma_start(md_t[:], madd_d[qq * 128:(qq + 1) * 128, :])
            qdat[qq] = (vt_t, md_t)

        def phase_b(qq, lm_t):
            vt_t, md_t = qdat.pop(qq)
            ladj = bp.tile([128, LP], F32, tag="ladj", name=f"ladj{qq}")
            nc.vector.tensor_tensor(ladj[:], lm_t[:], md_t[:], Alu.add)
            p_t = bp.tile([128, LP], BF16, tag="p", name=f"p{qq}")
            z_t = bp.tile([128, 1], F32, tag="z", name=f"z{qq}")
            nc.scalar.activation(p_t[:], ladj[:], Exp, accum_out=z_t[:])

            w1 = bp.tile([128, E * LP], BF16, tag="w1", name=f"w1{qq}")
            p_b = p_t[:].rearrange("p (o l) -> p o l", o=1).broadcast_to([128, E, LP])
            nc.vector.tensor_tensor(
                w1[:].rearrange("p (e l) -> p e l", e=E),
                vt_t[:].rearrange("p (e l) -> p e l", e=E),
                p_b, Alu.mult,
            )
            w2 = bp.tile([128, E * LP // 2], BF16, tag="w2", name=f"w2{qq}")
            w1v = w1[:].rearrange("p (e l) -> p e l", e=E)
            nc.vector.tensor_tensor(
                w2[:].rearrange("p (e l) -> p e l", e=E),
                w1v[:, :, 0:LP // 2], w1v[:, :, LP // 2:LP], Alu.add,
            )
            w3 = bp.tile([128, E * LP // 4], BF16, tag="w3", name=f"w3{qq}")
            w2v = w2[:].rearrange("p (e l) -> p e l", e=E)
            nc.vector.tensor_tensor(
                w3[:].rearrange("p (e l) -> p e l", e=E),
                w2v[:, :, 0:LP // 4], w2v[:, :, LP // 4:LP // 2], Alu.add,
            )
            acc = bp.tile([128, E], F32, tag="acc", name=f"acc{qq}")
            nc.vector.tensor_reduce(
                acc[:], w3[:].rearrange("p (e l) -> p e l", e=E), axis=X, op=Alu.add)
            rz = bp.tile([128, 1], F32, tag="rz", name=f"rz{qq}")
            nc.vector.reciprocal(rz[:], z_t[:])
            o_t = bp.tile([128, E], F32, tag="o", name=f"o{qq}")
            nc.vector.tensor_scalar_mul(o_t[:], acc[:], rz[:])
            nc.gpsimd.dma_start(out_d[qq * 128:(qq + 1) * 128, :], o_t[:])

        load_slab(0)
        st_t = None
        lg_t = None
        GP = 8  # pairs per h2 group ([80, 1024] f32 = 2 psum banks)
        slab_of_group = []
        for si, ng in enumerate(SLAB_GROUPS):
            slab_of_group += [si] * ng
        for g in range(32):
            s = slab_of_group[g]
            if g == SLAB_G0[s] and s + 1 < len(SLAB_GROUPS):
                load_slab(s + 1)
            if g == 2:
                load_quarter(0)
            if g % 8 == 4 and g // 8 + 1 < 4:
                load_quarter(g // 8 + 1)

            kxs, vws = kx_t[s], vw_t[s]
            h2_t = h2p.tile([80, GP * LP], F32, tag="h2", name=f"h2_{g}")
            for pp in range(GP):
                rr = (g - SLAB_G0[s]) * GP + pp  # pair within slab
                nc.tensor.matmul(
                    h2_t[0:80, pp * LP:(pp + 1) * LP],
                    vws[:, rr * 80:rr * 80 + 80],
                    kxs[:, rr * LP:(rr + 1) * LP],
                    start=True, stop=True,
                )
            t_t = tp.tile([80, GP * LP], BF16, tag="t", name=f"t_{g}")
            nc.scalar.activation(t_t[:], h2_t[:], Tanh, scale=0.5)

            if g % 2 == 0:
                lg_t = lgp.tile([98, 512], F32, tag="lg", name=f"lg_{g // 2}")
            for j in range(2):
                ss = 2 * (g % 2) + j
                nc.tensor.matmul(
                    lg_t[32 * ss:32 * ss + 2, 0:512],
                    wf_t[:], t_t[:, 512 * j:512 * (j + 1)],
                    start=True, stop=True, tile_position=(0, 32 * ss),
                )
            if g % 2 == 1:
                qq, gq = g // 8, (g // 2) % 4
                if gq == 0:
                    st_t = stp.tile([98, 4 * 512], BF16, tag="st", name=f"st{qq}")
                nc.scalar.activation(
                    st_t[:, 512 * gq:512 * (gq + 1)], lg_t[:], Copy)

                if gq == 3:
                    lm_t = lmp.tile([128, LP], BF16, tag="lm", name=f"lm{qq}")
                    for ss in range(4):
                        for sig in range(2):
                            row = 32 * ss + sig
                            dr = 16 * (2 * ss + sig)
                            # split across two queues: halves the serial
                            # dispatch latency ahead of phase B
                            eng = nc.sync if ss % 2 == 0 else nc.gpsimd
                            eng.dma_start(
                                lm_t[dr:dr + 16, :], st_t[row:row + 1, :])
                    phase_b(qq, lm_t)

    if not nc.is_finalized():
        nc.finalize()
    return nc


def host_prep(q, k, v, mask, W1, b1, W2, b2, Wf, bf):
    """Fold weights per batch, compact masked tokens, build device layouts."""
    q2 = q[:, 0, :].astype(np.float32)
    W1 = W1.astype(np.float32); W2 = W2.astype(np.float32)
    P = (W1[64:128] - W1[128:192]) @ W2                     # [64,40]
    Q = W1[192:256] @ W2                                    # [64,40]
    A2 = (W1[0:64] + W1[128:192]) @ W2
    c0 = b1.astype(np.float32) @ W2 + b2.astype(np.float32)
    r = q2 @ A2 + c0                                        # [Bf,40]
    V = P[None] + q2[:, :, None] * Q[None]                  # [Bf,64,40]
    G = np.einsum('beh,bei->bhi', V, V)
    y = np.linalg.solve(G, r[:, :, None])
    s = np.einsum('beh,bhx->be', V, y)                      # [Bf,64]

    m = mask[:, :, 0]
    order = np.argsort(m, axis=1, kind='stable')[:, :LP]
    nvalid = (~m).sum(1)
    assert nvalid.max() <= LP, f"batch with {nvalid.max()} unmasked tokens"
    validc = np.arange(LP)[None, :] < nvalid[:, None]       # [Bf,LP]
    kc = np.take_along_axis(k.astype(np.float32), order[:, :, None], 1)
    vc = np.take_along_axis(v.astype(np.float32), order[:, :, None], 1)
    kc = np.where(validc[..., None], kc + s[:, None, :], 0.0)
    vc = np.where(validc[..., None], vc, 0.0)
    maddf = np.where(validc, np.float32(0.0), np.float32(-30.0)).astype(nbf16)

    # core-local batch <-> (pair r, stream sig) map
    b = np.arange(B)
    qq = b // 128; t = b % 128
    s2s = t // 16; s_ = s2s // 2; sig = s2s % 2
    g_ = (t % 16) // 4; cb = t % 4
    r_ = 64 * qq + 16 * g_ + 4 * s_ + cb
    A_idx = np.empty(NPAIR, np.int64); B_idx = np.empty(NPAIR, np.int64)
    A_idx[r_[sig == 0]] = b[sig == 0]
    B_idx[r_[sig == 1]] = b[sig == 1]

    in_maps = []
    for c in range(N_CORES):
        sl = slice(c * B, (c + 1) * B)
        kcc, Vc = kc[sl], V[sl]
        kx = np.empty((128, NPAIR * LP), np.float32)
        kx[0:64] = kcc[A_idx].transpose(2, 0, 1).reshape(64, -1)
        kx[64:128] = kcc[B_idx].transpose(2, 0, 1).reshape(64, -1)
        vw3 = np.zeros((NPAIR, 128, 80), np.float32)
        vw3[:, 0:64, 0:40] = Vc[A_idx]
        vw3[:, 64:128, 40:80] = Vc[B_idx]
        vw = vw3.transpose(1, 0, 2).reshape(128, NPAIR * 80)
        vt = np.ascontiguousarray(vc[sl].transpose(0, 2, 1)).reshape(B, E * LP)
        wf2 = np.zeros((80, 2), np.float32)
        wf2[0:40, 0] = 0.5 * Wf[:, 0]
        wf2[40:80, 1] = 0.5 * Wf[:, 0]
        in_maps.append({
            "kx": np.ascontiguousarray(kx).astype(nbf16),
            "vw": np.ascontiguousarray(vw).astype(nbf16),
            "wf2": wf2.astype(nbf16),
            "vt": vt.astype(nbf16),
            "madd": np.ascontiguousarray(maddf[sl]),
        })
    return in_maps


_CACHE = {}


def run_on_device(in_maps, trace=False):
    if "nc" not in _CACHE:
        _CACHE["nc"] = build_nc()
    nc = _CACHE["nc"]
    res = run_bass_kernel_spmd(nc, in_maps, core_ids=list(range(N_CORES)),
                               trace=trace)
    return res


def kernel(q, k, v, mask, W1, b1, W2, b2, Wf, bf):
    in_maps = host_prep(q, k, v, mask, W1, b1, W2, b2, Wf, bf)
    res = run_on_device(in_maps)
    out = np.concatenate([res.results[c]["out"] for c in range(N_CORES)], axis=0)
    return out.astype(np.float32)
